# revision 19
# baseline (speedup 1.0000x reference)
"""CrossAttention2D Trainium2 Bass kernel.

Problem (per batch item b, C=128, HW=64*64=4096):
    q = Wq @ xq + bq            # [C, HW]   (1x1 conv == GEMM)
    k = Wk @ xk + bk            # [C, HW]
    S = (q^T k) / sqrt(HW)      # [HW, HW]
    A = softmax(S, axis=-1)
    out = (A @ v^T)^T + q       # [C, HW],  v = xv

Sharding: data-parallel over batch B=8 -> one batch item per NeuronCore.

Per-core algorithm (no collectives):
  - Q/K proj in fp32 (Q feeds the residual directly); q/k cast to bf16
    for the score matmuls.
  - V transposed on the PE to vT[tk, c] (bf16) with a ones column
    (col 128) so the PV matmul accumulates the softmax denominator free.
  - Scores computed TRANSPOSED: S^T tiles [tk=128, tq=1024] spanning 2
    PSUM banks; ScalarE evacuates with exp(S/64) in one FD=1024 ACT op
    (softmax without max-subtraction: |S| <= ~1.2 for randn inputs).
  - PV: out_ext[tq,129] += expS^T_slice^T @ vT_ext over 32 tk blocks,
    PSUM-accumulated, 3 accumulator groups packed per PSUM bank (a
    zero-matmul initializes each bank since start=True clears it whole).
  - Finalize (software-pipelined into the next chunk so ACT never
    idles): DVE normalize, PE transpose back to [c, tq], DVE residual
    add, DMA out.

Engine budget per core: ACT ~136us exp (bottleneck), PE ~90us, DVE ~35us.
"""

import os
import numpy as np

B, C, H, W = 8, 128, 64, 64
HW = H * W            # 4096
P = 128
TQ = 512              # moving free dim of one S^T matmul (PSUM bank width)
TQC = 1024            # query-token chunk (2 banks wide -> one FD=1024 exp)
NCHUNK = HW // TQC    # 4
NTK = HW // P         # 32 key blocks
VT_STRIDE = 130       # 129 used + 1 pad to keep 4B alignment per block
PREF = 4              # S/exp groups emitted before the previous finalize
OPACK = 3             # accumulator groups packed per PSUM bank

_CACHE: dict = {}
LAST_RESULTS = None   # BassKernelResults of the most recent run (for test.py)


def _build_kernel():
    import concourse.tile as tile
    from concourse import bacc, mybir
    from concourse.masks import make_identity

    f32 = mybir.dt.float32
    bf16 = mybir.dt.bfloat16
    AF = mybir.ActivationFunctionType

    nc = bacc.Bacc("TRN2", target_bir_lowering=False, debug=False)

    xq = nc.dram_tensor("xq", [C, HW], f32, kind="ExternalInput")
    xk = nc.dram_tensor("xk", [C, HW], f32, kind="ExternalInput")
    xv = nc.dram_tensor("xv", [C, HW], f32, kind="ExternalInput")
    wqT = nc.dram_tensor("wqT", [C, C], f32, kind="ExternalInput")
    wkT = nc.dram_tensor("wkT", [C, C], f32, kind="ExternalInput")
    bqv = nc.dram_tensor("bqv", [C, 1], f32, kind="ExternalInput")
    bkv = nc.dram_tensor("bkv", [C, 1], f32, kind="ExternalInput")
    out = nc.dram_tensor("out", [C, HW], f32, kind="ExternalOutput")

    inv_sqrt_hw = 1.0 / float(np.sqrt(HW))

    with tile.TileContext(nc) as tc:
        with (
            tc.tile_pool(name="const", bufs=1) as cpool,
            tc.tile_pool(name="stage", bufs=1) as spool,
            tc.tile_pool(name="expp", bufs=8) as epool,
            tc.tile_pool(name="fin", bufs=3) as fpool,
            tc.tile_pool(name="ps_s", bufs=2, space="PSUM") as pss,
        ):
            # ---------- constants / weights ----------
            wq_sb = cpool.tile([C, C], f32, name="wq_sb")
            wk_sb = cpool.tile([C, C], f32, name="wk_sb")
            bq_sb = cpool.tile([C, 1], f32, name="bq_sb")
            bk_sb = cpool.tile([C, 1], f32, name="bk_sb")
            ident_f = cpool.tile([P, P], f32, name="ident_f")
            zeros_b = cpool.tile([P, OPACK * 129], bf16, name="zeros_b")
            nc.sync.dma_start(wq_sb[:], wqT[:])
            nc.sync.dma_start(wk_sb[:], wkT[:])
            nc.sync.dma_start(bq_sb[:], bqv[:])
            nc.sync.dma_start(bk_sb[:], bkv[:])
            make_identity(nc, ident_f)
            nc.gpsimd.memset(zeros_b[:], 0.0)

            # ---------- input staging ----------
            # DMA order = dependency-chain length: xv feeds the V-transpose
            # chain, xq[:TQC] + xk[:TQ..] feed the first score tiles; xq's
            # tail is only needed a full chunk later.
            xq_sb = spool.tile([C, HW], f32, name="xq_sb")
            xk_sb = spool.tile([C, HW], f32, name="xk_sb")
            xv_sb = spool.tile([C, HW], f32, name="xv_sb")
            for j in range(HW // TQ):
                nc.sync.dma_start(xv_sb[:, j * TQ:(j + 1) * TQ],
                                  xv[:, j * TQ:(j + 1) * TQ])
            for j in range(TQC // TQ):
                nc.sync.dma_start(xq_sb[:, j * TQ:(j + 1) * TQ],
                                  xq[:, j * TQ:(j + 1) * TQ])
            for j in range(HW // TQ):
                nc.sync.dma_start(xk_sb[:, j * TQ:(j + 1) * TQ],
                                  xk[:, j * TQ:(j + 1) * TQ])
            for j in range(TQC // TQ, HW // TQ):
                nc.sync.dma_start(xq_sb[:, j * TQ:(j + 1) * TQ],
                                  xq[:, j * TQ:(j + 1) * TQ])

            # ---------- projections (bias add + PSUM evac on DVE) ----------
            q_f32 = spool.tile([C, HW], f32, name="q_f32")
            q_bf = spool.tile([C, HW], bf16, name="q_bf")
            k_bf = spool.tile([C, HW], bf16, name="k_bf")

            def q_proj(j):
                sl = slice(j * TQ, (j + 1) * TQ)
                qp = pss.tile([P, TQ], f32, name="qp", tag="ps")
                nc.tensor.matmul(qp[:], wq_sb[:], xq_sb[:, sl],
                                 start=True, stop=True)
                nc.vector.tensor_scalar_add(q_f32[:, sl], qp[:], bq_sb[:])
                nc.vector.tensor_copy(q_bf[:, sl], q_f32[:, sl])

            def k_proj(j):
                sl = slice(j * TQ, (j + 1) * TQ)
                kp = pss.tile([P, TQ], f32, name="kp", tag="ps")
                nc.tensor.matmul(kp[:], wk_sb[:], xk_sb[:, sl],
                                 start=True, stop=True)
                nc.vector.tensor_scalar_add(k_bf[:, sl], kp[:], bk_sb[:])

            q_proj(0)
            q_proj(1)

            # ---------- V transpose (vT_ext with ones column) ----------
            # Own PSUM pool (released before the PV pool opens); evacuation
            # alternates ScalarE/VectorE, both idle this early.
            vt = spool.tile([P, NTK, VT_STRIDE], bf16, name="vt")
            nc.gpsimd.memset(vt[:, :, 128:129], 1.0)
            with tc.tile_pool(name="ps_vt", bufs=4, space="PSUM") as pvt:
                for blk in range(NTK):
                    tp = pvt.tile([P, P], f32, name="vtp", tag="vtp")
                    nc.tensor.transpose(tp[:], xv_sb[:, blk * P:(blk + 1) * P],
                                        ident_f[:])
                    if blk % 2 == 0:
                        nc.scalar.copy(vt[:, blk, 0:128], tp[:])
                    else:
                        nc.vector.tensor_copy(vt[:, blk, 0:128], tp[:])

            k_proj(0)
            k_proj(1)
            k_done = 2
            q_done = 2

            # ---------- attention main loop ----------
            pso = tc.alloc_tile_pool(name="ps_o", bufs=OPACK, space="PSUM")
            pst = tc.alloc_tile_pool(name="ps_t", bufs=1, space="PSUM")

            def emit_s_exp(chunk, blk):
                s_ps = pss.tile([P, TQC], f32, name="s_ps", tag="ps")
                for h in range(TQC // TQ):
                    nc.tensor.matmul(
                        s_ps[:, h * TQ:(h + 1) * TQ],
                        k_bf[:, blk * P:(blk + 1) * P],
                        q_bf[:, chunk * TQC + h * TQ:
                             chunk * TQC + (h + 1) * TQ],
                        start=True, stop=True)
                e_sb = epool.tile([P, TQC], bf16, name="e_sb", tag="exp")
                nc.scalar.activation(e_sb[:], s_ps[:], AF.Exp,
                                     scale=inv_sqrt_hw)
                return e_sb

            def emit_pv(o_tiles, e_sb, blk):
                for j in range(8):
                    nc.tensor.matmul(o_tiles[j // OPACK][:, j % OPACK, 0:129],
                                     e_sb[:, j * P:(j + 1) * P],
                                     vt[:, blk, 0:129],
                                     start=False, stop=(blk == NTK - 1),
                                     skip_group_check=True)

            def emit_finalize(chunk, o_tiles):
                an_tiles = []
                for j in range(8):
                    o_ap = o_tiles[j // OPACK][:, j % OPACK, :]
                    rec = fpool.tile([P, 1], f32, name="rec", tag="rec",
                                     bufs=8)
                    nc.vector.reciprocal(rec[:], o_ap[:, 128:129])
                    an = fpool.tile([P, P], f32, name="an", tag="an", bufs=8)
                    nc.vector.tensor_scalar_mul(an[:], o_ap[:, 0:128], rec[:])
                    an_tiles.append(an)
                for j in range(8):
                    tq0 = chunk * TQC + j * P
                    tp2 = pst.tile([P, P], f32, name="tp2", tag="t")
                    nc.tensor.transpose(tp2[:], an_tiles[j][:], ident_f[:])
                    ob = fpool.tile([P, P], f32, name="ob", tag="ob", bufs=4)
                    nc.vector.tensor_add(ob[:], tp2[:],
                                         q_f32[:, tq0:tq0 + P])
                    nc.sync.dma_start(out[:, tq0:tq0 + P], ob[:])

            pending = None
            for chunk in range(NCHUNK):
                npref = PREF if pending is not None else 0
                pre = []
                for blk in range(npref):
                    pre.append(emit_s_exp(chunk, blk))
                if pending is not None:
                    emit_finalize(*pending)
                    pending = None
                ngroups = (8 + OPACK - 1) // OPACK
                o_tiles = [
                    pso.tile([P, OPACK, 129], f32, name="o_ps", tag="o")
                    for _ in range(ngroups)
                ]
                # start=True clears the whole bank, so packed accumulation
                # groups can't each own a start; one zero-matmul inits each.
                for t in range(ngroups):
                    nc.tensor.matmul(o_tiles[t][:, :, :],
                                     zeros_b[:, 0:128], zeros_b[:],
                                     start=True, stop=False,
                                     skip_group_check=True)
                for blk in range(npref):
                    emit_pv(o_tiles, pre[blk], blk)
                for blk in range(npref, NTK):
                    # chunk 0: feed K projections just ahead of their use
                    if chunk == 0 and blk % 4 == 0 and k_done < HW // TQ:
                        k_proj(k_done)
                        k_done += 1
                    if blk == 16 and q_done < 2 * (chunk + 2):
                        while q_done < min(2 * (chunk + 2), HW // TQ):
                            q_proj(q_done)
                            q_done += 1
                    e_sb = emit_s_exp(chunk, blk)
                    emit_pv(o_tiles, e_sb, blk)
                pending = (chunk, o_tiles)
            emit_finalize(*pending)
            pst.release()
            pso.release()

    nc.finalize()
    return nc


def kernel(query_img, key_img, value_img, Wq, bq, Wk, bk):
    from concourse.bass_utils import run_bass_kernel_spmd

    global LAST_RESULTS

    query_img = np.asarray(query_img, dtype=np.float32)
    key_img = np.asarray(key_img, dtype=np.float32)
    value_img = np.asarray(value_img, dtype=np.float32)
    wqT = np.ascontiguousarray(np.asarray(Wq, dtype=np.float32).T)
    wkT = np.ascontiguousarray(np.asarray(Wk, dtype=np.float32).T)
    bqc = np.ascontiguousarray(np.asarray(bq, dtype=np.float32).reshape(C, 1))
    bkc = np.ascontiguousarray(np.asarray(bk, dtype=np.float32).reshape(C, 1))

    if "nc" not in _CACHE:
        _CACHE["nc"] = _build_kernel()
    nc = _CACHE["nc"]

    in_maps = []
    for b in range(B):
        in_maps.append({
            "xq": np.ascontiguousarray(query_img[b].reshape(C, HW)),
            "xk": np.ascontiguousarray(key_img[b].reshape(C, HW)),
            "xv": np.ascontiguousarray(value_img[b].reshape(C, HW)),
            "wqT": wqT,
            "wkT": wkT,
            "bqv": bqc,
            "bkv": bkc,
        })

    trace = os.environ.get("KERNEL_TRACE", "0") == "1"
    res = run_bass_kernel_spmd(nc, in_maps, core_ids=list(range(B)),
                               trace=trace)
    LAST_RESULTS = res
    out = np.stack([res.results[b]["out"].reshape(C, H, W) for b in range(B)])
    return out.astype(np.float32)


# revision 25
# speedup vs baseline: 1.0399x; 1.0399x over previous
"""CrossAttention2D Trainium2 Bass kernel.

Problem (per batch item b, C=128, HW=64*64=4096):
    q = Wq @ xq + bq            # [C, HW]   (1x1 conv == GEMM)
    k = Wk @ xk + bk            # [C, HW]
    S = (q^T k) / sqrt(HW)      # [HW, HW]
    A = softmax(S, axis=-1)
    out = (A @ v^T)^T + q       # [C, HW],  v = xv

Sharding: data-parallel over batch B=8 -> one batch item per NeuronCore.

Per-core algorithm (no collectives):
  - Q/K proj in fp32 (Q feeds the residual directly); q/k cast to bf16
    for the score matmuls.
  - V transposed on the PE to vT[tk, c] (bf16) with a ones column
    (col 128) so the PV matmul accumulates the softmax denominator free.
  - Scores computed TRANSPOSED: S^T tiles [tk=128, tq=1024] spanning 2
    PSUM banks; ScalarE evacuates with exp(S/64) in one FD=1024 ACT op
    (softmax without max-subtraction: |S| <= ~1.2 for randn inputs).
  - PV: out_ext[tq,129] += expS^T_slice^T @ vT_ext over 32 tk blocks,
    PSUM-accumulated, 3 accumulator groups packed per PSUM bank (a
    zero-matmul initializes each bank since start=True clears it whole).
  - Finalize (software-pipelined into the next chunk so ACT never
    idles): DVE normalize, PE transpose back to [c, tq], DVE residual
    add, DMA out.

Engine budget per core: ACT ~136us exp (bottleneck), PE ~90us, DVE ~35us.
"""

import os
import numpy as np

B, C, H, W = 8, 128, 64, 64
HW = H * W            # 4096
P = 128
TQ = 512              # moving free dim of one S^T matmul (PSUM bank width)
TQC = 1024            # query-token chunk (2 banks wide -> one FD=1024 exp)
NCHUNK = HW // TQC    # 4
NTK = HW // P         # 32 key blocks
VT_STRIDE = 130       # 129 used + 1 pad to keep 4B alignment per block
PREF = 6              # S/exp groups emitted before the previous finalize
OPACK = 3             # accumulator groups packed per PSUM bank

_CACHE: dict = {}
LAST_RESULTS = None   # BassKernelResults of the most recent run (for test.py)


def _build_kernel():
    import concourse.tile as tile
    from concourse import bacc, mybir
    from concourse.masks import make_identity

    f32 = mybir.dt.float32
    bf16 = mybir.dt.bfloat16
    AF = mybir.ActivationFunctionType

    nc = bacc.Bacc("TRN2", target_bir_lowering=False, debug=False)

    xq = nc.dram_tensor("xq", [C, HW], f32, kind="ExternalInput")
    xk = nc.dram_tensor("xk", [C, HW], f32, kind="ExternalInput")
    xv = nc.dram_tensor("xv", [C, HW], f32, kind="ExternalInput")
    wqT = nc.dram_tensor("wqT", [C, C], f32, kind="ExternalInput")
    wkT = nc.dram_tensor("wkT", [C, C], f32, kind="ExternalInput")
    bqv = nc.dram_tensor("bqv", [C, 1], f32, kind="ExternalInput")
    bkv = nc.dram_tensor("bkv", [C, 1], f32, kind="ExternalInput")
    out = nc.dram_tensor("out", [C, HW], f32, kind="ExternalOutput")

    inv_sqrt_hw = 1.0 / float(np.sqrt(HW))

    with tile.TileContext(nc) as tc:
        with (
            tc.tile_pool(name="const", bufs=1) as cpool,
            tc.tile_pool(name="stage", bufs=1) as spool,
            tc.tile_pool(name="expp", bufs=10) as epool,
            tc.tile_pool(name="fin", bufs=3) as fpool,
            tc.tile_pool(name="ps_s", bufs=2, space="PSUM") as pss,
        ):
            # ---------- constants / weights ----------
            wq_sb = cpool.tile([C, C], f32, name="wq_sb")
            wk_sb = cpool.tile([C, C], f32, name="wk_sb")
            bq_sb = cpool.tile([C, 1], f32, name="bq_sb")
            bk_sb = cpool.tile([C, 1], f32, name="bk_sb")
            ident_f = cpool.tile([P, P], f32, name="ident_f")
            zeros_b = cpool.tile([P, OPACK * 129], bf16, name="zeros_b")
            nc.sync.dma_start(wq_sb[:], wqT[:])
            nc.sync.dma_start(wk_sb[:], wkT[:])
            nc.sync.dma_start(bq_sb[:], bqv[:])
            nc.sync.dma_start(bk_sb[:], bkv[:])
            make_identity(nc, ident_f)
            nc.gpsimd.memset(zeros_b[:], 0.0)

            # ---------- input staging ----------
            # DMA order = dependency-chain length: xv feeds the V-transpose
            # chain, xq[:TQC] + xk[:TQ..] feed the first score tiles; xq's
            # tail is only needed a full chunk later.
            xq_sb = spool.tile([C, HW], f32, name="xq_sb")
            xk_sb = spool.tile([C, HW], f32, name="xk_sb")
            xv_sb = spool.tile([C, HW], f32, name="xv_sb")
            for j in range(TQC // TQ):
                nc.sync.dma_start(xq_sb[:, j * TQ:(j + 1) * TQ],
                                  xq[:, j * TQ:(j + 1) * TQ])
            nc.sync.dma_start(xk_sb[:, 0:TQ], xk[:, 0:TQ])
            for j in range(HW // TQ):
                nc.sync.dma_start(xv_sb[:, j * TQ:(j + 1) * TQ],
                                  xv[:, j * TQ:(j + 1) * TQ])
            for j in range(1, HW // TQ):
                nc.sync.dma_start(xk_sb[:, j * TQ:(j + 1) * TQ],
                                  xk[:, j * TQ:(j + 1) * TQ])
            for j in range(TQC // TQ, HW // TQ):
                nc.sync.dma_start(xq_sb[:, j * TQ:(j + 1) * TQ],
                                  xq[:, j * TQ:(j + 1) * TQ])

            # ---------- projections (bias add + PSUM evac on DVE) ----------
            q_f32 = spool.tile([C, HW], f32, name="q_f32")
            q_bf = spool.tile([C, HW], bf16, name="q_bf")
            k_bf = spool.tile([C, HW], bf16, name="k_bf")

            # Later projections run through the single-bank "t" pool so they
            # never steal a slot from the 2-deep score ring mid-stream;
            # startup projections use the still-idle score ring.
            pst = tc.alloc_tile_pool(name="ps_t", bufs=1, space="PSUM")

            def q_proj(j, pool, tag):
                sl = slice(j * TQ, (j + 1) * TQ)
                qp = pool.tile([P, TQ], f32, name="qp", tag=tag)
                nc.tensor.matmul(qp[:], wq_sb[:], xq_sb[:, sl],
                                 start=True, stop=True)
                nc.vector.tensor_scalar_add(q_f32[:, sl], qp[:], bq_sb[:])
                nc.vector.tensor_copy(q_bf[:, sl], q_f32[:, sl])

            def k_proj(j, pool, tag):
                sl = slice(j * TQ, (j + 1) * TQ)
                kp = pool.tile([P, TQ], f32, name="kp", tag=tag)
                nc.tensor.matmul(kp[:], wk_sb[:], xk_sb[:, sl],
                                 start=True, stop=True)
                nc.vector.tensor_scalar_add(k_bf[:, sl], kp[:], bk_sb[:])

            q_proj(0, pss, "ps")
            q_proj(1, pss, "ps")
            k_proj(0, pss, "ps")
            k_done = 1
            q_done = 2

            vt = spool.tile([P, NTK, VT_STRIDE], bf16, name="vt")

            def emit_s_exp(chunk, blk):
                s_ps = pss.tile([P, TQC], f32, name="s_ps", tag="ps")
                for h in range(TQC // TQ):
                    nc.tensor.matmul(
                        s_ps[:, h * TQ:(h + 1) * TQ],
                        k_bf[:, blk * P:(blk + 1) * P],
                        q_bf[:, chunk * TQC + h * TQ:
                             chunk * TQC + (h + 1) * TQ],
                        start=True, stop=True)
                e_sb = epool.tile([P, TQC], bf16, name="e_sb", tag="exp")
                nc.scalar.activation(e_sb[:], s_ps[:], AF.Exp,
                                     scale=inv_sqrt_hw)
                return e_sb

            def emit_pv(o_tiles, e_sb, blk):
                for j in range(8):
                    nc.tensor.matmul(o_tiles[j // OPACK][:, j % OPACK, 0:129],
                                     e_sb[:, j * P:(j + 1) * P],
                                     vt[:, blk, 0:129],
                                     start=False, stop=(blk == NTK - 1),
                                     skip_group_check=True)

            def emit_finalize(chunk, o_tiles):
                an_tiles = []
                for j in range(8):
                    o_ap = o_tiles[j // OPACK][:, j % OPACK, :]
                    rec = fpool.tile([P, 1], f32, name="rec", tag="rec",
                                     bufs=8)
                    nc.vector.reciprocal(rec[:], o_ap[:, 128:129])
                    an = fpool.tile([P, P], f32, name="an", tag="an", bufs=8)
                    nc.vector.tensor_scalar_mul(an[:], o_ap[:, 0:128], rec[:])
                    an_tiles.append(an)
                for j in range(8):
                    tq0 = chunk * TQC + j * P
                    tp2 = pst.tile([P, P], f32, name="tp2", tag="t")
                    nc.tensor.transpose(tp2[:], an_tiles[j][:], ident_f[:])
                    ob = fpool.tile([P, P], f32, name="ob", tag="ob", bufs=4)
                    nc.vector.tensor_add(ob[:], tp2[:],
                                         q_f32[:, tq0:tq0 + P])
                    nc.sync.dma_start(out[:, tq0:tq0 + P], ob[:])

            def alloc_o_tiles():
                ngroups = (8 + OPACK - 1) // OPACK
                o_tiles = [
                    pso.tile([P, OPACK, 129], f32, name="o_ps", tag="o")
                    for _ in range(ngroups)
                ]
                # start=True clears the whole bank, so packed accumulation
                # groups can't each own a start; one zero-matmul inits each.
                for t in range(ngroups):
                    nc.tensor.matmul(o_tiles[t][:, :, :],
                                     zeros_b[:, 0:128], zeros_b[:],
                                     start=True, stop=False,
                                     skip_group_check=True)
                return o_tiles

            # ---- chunk 0 head: first score/exp groups need only k block
            # 0..3 (k_proj 0), so they are emitted BEFORE the V transposes —
            # ScalarE starts the exp stream while vT is still being built.
            pre0 = [emit_s_exp(0, blk) for blk in range(4)]

            # ---- V transpose (vT_ext with ones column), feeding k_proj
            # just ahead of use; evacuation alternates ScalarE/VectorE.
            nc.gpsimd.memset(vt[:, :, 128:129], 1.0)
            with tc.tile_pool(name="ps_vt", bufs=3, space="PSUM") as pvt:
                for blk in range(NTK):
                    tp = pvt.tile([P, P], f32, name="vtp", tag="vtp")
                    nc.tensor.transpose(tp[:], xv_sb[:, blk * P:(blk + 1) * P],
                                        ident_f[:])
                    if blk % 2 == 0:
                        nc.scalar.copy(vt[:, blk, 0:128], tp[:])
                    else:
                        nc.vector.tensor_copy(vt[:, blk, 0:128], tp[:])
                    if blk % 4 == 3 and k_done < HW // TQ:
                        k_proj(k_done, pst, "t")
                        k_done += 1

            pso = tc.alloc_tile_pool(name="ps_o", bufs=OPACK, space="PSUM")

            pending = None
            for chunk in range(NCHUNK):
                npref = 4 if chunk == 0 else PREF
                pre = pre0 if chunk == 0 else \
                    [emit_s_exp(chunk, blk) for blk in range(npref)]
                if pending is not None:
                    emit_finalize(*pending)
                    pending = None
                o_tiles = alloc_o_tiles()
                for blk in range(npref):
                    emit_pv(o_tiles, pre[blk], blk)
                for blk in range(npref, NTK):
                    # feed next chunks' q projections, one insertion at a
                    # time so the PE-FIFO bubble stays under the exp buffer
                    if blk in (12, 20) and q_done < min(2 * (chunk + 2),
                                                        HW // TQ):
                        q_proj(q_done, pst, "t")
                        q_done += 1
                    e_sb = emit_s_exp(chunk, blk)
                    emit_pv(o_tiles, e_sb, blk)
                pending = (chunk, o_tiles)
            emit_finalize(*pending)
            pso.release()
            pst.release()

    nc.finalize()
    return nc


def kernel(query_img, key_img, value_img, Wq, bq, Wk, bk):
    from concourse.bass_utils import run_bass_kernel_spmd

    global LAST_RESULTS

    query_img = np.asarray(query_img, dtype=np.float32)
    key_img = np.asarray(key_img, dtype=np.float32)
    value_img = np.asarray(value_img, dtype=np.float32)
    wqT = np.ascontiguousarray(np.asarray(Wq, dtype=np.float32).T)
    wkT = np.ascontiguousarray(np.asarray(Wk, dtype=np.float32).T)
    bqc = np.ascontiguousarray(np.asarray(bq, dtype=np.float32).reshape(C, 1))
    bkc = np.ascontiguousarray(np.asarray(bk, dtype=np.float32).reshape(C, 1))

    if "nc" not in _CACHE:
        _CACHE["nc"] = _build_kernel()
    nc = _CACHE["nc"]

    in_maps = []
    for b in range(B):
        in_maps.append({
            "xq": np.ascontiguousarray(query_img[b].reshape(C, HW)),
            "xk": np.ascontiguousarray(key_img[b].reshape(C, HW)),
            "xv": np.ascontiguousarray(value_img[b].reshape(C, HW)),
            "wqT": wqT,
            "wkT": wkT,
            "bqv": bqc,
            "bkv": bkc,
        })

    trace = os.environ.get("KERNEL_TRACE", "0") == "1"
    res = run_bass_kernel_spmd(nc, in_maps, core_ids=list(range(B)),
                               trace=trace)
    LAST_RESULTS = res
    out = np.stack([res.results[b]["out"].reshape(C, H, W) for b in range(B)])
    return out.astype(np.float32)


# revision 27
# speedup vs baseline: 1.0793x; 1.0379x over previous
"""CrossAttention2D Trainium2 Bass kernel.

Problem (per batch item b, C=128, HW=64*64=4096):
    q = Wq @ xq + bq            # [C, HW]   (1x1 conv == GEMM)
    k = Wk @ xk + bk            # [C, HW]
    S = (q^T k) / sqrt(HW)      # [HW, HW]
    A = softmax(S, axis=-1)
    out = (A @ v^T)^T + q       # [C, HW],  v = xv

Sharding: data-parallel over batch B=8 -> one batch item per NeuronCore.

Per-core algorithm (no collectives):
  - Q/K proj in fp32 (Q feeds the residual directly); q/k cast to bf16
    for the score matmuls.
  - V transposed on the PE to vT[tk, c] (bf16) with a ones column
    (col 128) so the PV matmul accumulates the softmax denominator free.
  - Scores computed TRANSPOSED: S^T tiles [tk=128, tq=1024] spanning 2
    PSUM banks; ScalarE evacuates with exp(S/64) in one FD=1024 ACT op
    (softmax without max-subtraction: |S| <= ~1.2 for randn inputs).
  - PV: out_ext[tq,129] += expS^T_slice^T @ vT_ext over 32 tk blocks,
    PSUM-accumulated, 3 accumulator groups packed per PSUM bank (a
    zero-matmul initializes each bank since start=True clears it whole).
  - Finalize (software-pipelined into the next chunk so ACT never
    idles): DVE normalize, PE transpose back to [c, tq], DVE residual
    add, DMA out.

Engine budget per core: ACT ~136us exp (bottleneck), PE ~90us, DVE ~35us.
"""

import os
import numpy as np

B, C, H, W = 8, 128, 64, 64
HW = H * W            # 4096
P = 128
TQ = 512              # moving free dim of one S^T matmul (PSUM bank width)
TQC = 1024            # query-token chunk (2 banks wide -> one FD=1024 exp)
NCHUNK = HW // TQC    # 4
NTK = HW // P         # 32 key blocks
VT_STRIDE = 130       # 129 used + 1 pad to keep 4B alignment per block
PREF = 7              # S/exp groups emitted before the previous finalize
OPACK = 3             # accumulator groups packed per PSUM bank

_CACHE: dict = {}
LAST_RESULTS = None   # BassKernelResults of the most recent run (for test.py)


def _build_kernel():
    import concourse.tile as tile
    from concourse import bacc, mybir
    from concourse.masks import make_identity

    f32 = mybir.dt.float32
    bf16 = mybir.dt.bfloat16
    AF = mybir.ActivationFunctionType

    nc = bacc.Bacc("TRN2", target_bir_lowering=False, debug=False)

    xq = nc.dram_tensor("xq", [C, HW], f32, kind="ExternalInput")
    xk = nc.dram_tensor("xk", [C, HW], f32, kind="ExternalInput")
    xv = nc.dram_tensor("xv", [C, HW], f32, kind="ExternalInput")
    wqT = nc.dram_tensor("wqT", [C, C], f32, kind="ExternalInput")
    wkT = nc.dram_tensor("wkT", [C, C], f32, kind="ExternalInput")
    bqv = nc.dram_tensor("bqv", [C, 1], f32, kind="ExternalInput")
    bkv = nc.dram_tensor("bkv", [C, 1], f32, kind="ExternalInput")
    out = nc.dram_tensor("out", [C, HW], f32, kind="ExternalOutput")

    inv_sqrt_hw = 1.0 / float(np.sqrt(HW))

    with tile.TileContext(nc) as tc:
        with (
            tc.tile_pool(name="const", bufs=1) as cpool,
            tc.tile_pool(name="stage", bufs=1) as spool,
            tc.tile_pool(name="expp", bufs=10) as epool,
            tc.tile_pool(name="fin", bufs=3) as fpool,
            tc.tile_pool(name="ps_s", bufs=2, space="PSUM") as pss,
        ):
            # ---------- constants / weights ----------
            wq_sb = cpool.tile([C, C], f32, name="wq_sb")
            wk_sb = cpool.tile([C, C], f32, name="wk_sb")
            bq_sb = cpool.tile([C, 1], f32, name="bq_sb")
            bk_sb = cpool.tile([C, 1], f32, name="bk_sb")
            ident_f = cpool.tile([P, P], f32, name="ident_f")
            zeros_b = cpool.tile([P, OPACK * 129], bf16, name="zeros_b")
            nc.sync.dma_start(wq_sb[:], wqT[:])
            nc.sync.dma_start(wk_sb[:], wkT[:])
            nc.sync.dma_start(bq_sb[:], bqv[:])
            nc.sync.dma_start(bk_sb[:], bkv[:])
            make_identity(nc, ident_f)
            nc.gpsimd.memset(zeros_b[:], 0.0)

            # ---------- input staging ----------
            # DMA order = dependency-chain length: xv feeds the V-transpose
            # chain, xq[:TQC] + xk[:TQ..] feed the first score tiles; xq's
            # tail is only needed a full chunk later.
            xq_sb = spool.tile([C, HW], f32, name="xq_sb")
            xk_sb = spool.tile([C, HW], f32, name="xk_sb")
            xv_sb = spool.tile([C, HW], f32, name="xv_sb")
            for j in range(TQC // TQ):
                nc.sync.dma_start(xq_sb[:, j * TQ:(j + 1) * TQ],
                                  xq[:, j * TQ:(j + 1) * TQ])
            nc.sync.dma_start(xk_sb[:, 0:TQ], xk[:, 0:TQ])
            nc.sync.dma_start(xk_sb[:, TQ:2 * TQ], xk[:, TQ:2 * TQ])
            for j in range(HW // TQ):
                nc.sync.dma_start(xv_sb[:, j * TQ:(j + 1) * TQ],
                                  xv[:, j * TQ:(j + 1) * TQ])
            for j in range(2, HW // TQ):
                nc.sync.dma_start(xk_sb[:, j * TQ:(j + 1) * TQ],
                                  xk[:, j * TQ:(j + 1) * TQ])
            for j in range(TQC // TQ, HW // TQ):
                nc.sync.dma_start(xq_sb[:, j * TQ:(j + 1) * TQ],
                                  xq[:, j * TQ:(j + 1) * TQ])

            # ---------- projections (bias add + PSUM evac on DVE) ----------
            q_f32 = spool.tile([C, HW], f32, name="q_f32")
            q_bf = spool.tile([C, HW], bf16, name="q_bf")
            k_bf = spool.tile([C, HW], bf16, name="k_bf")

            # Later projections run through the single-bank "t" pool so they
            # never steal a slot from the 2-deep score ring mid-stream;
            # startup projections use the still-idle score ring.
            pst = tc.alloc_tile_pool(name="ps_t", bufs=1, space="PSUM")

            def q_proj(j, pool, tag):
                sl = slice(j * TQ, (j + 1) * TQ)
                qp = pool.tile([P, TQ], f32, name="qp", tag=tag)
                nc.tensor.matmul(qp[:], wq_sb[:], xq_sb[:, sl],
                                 start=True, stop=True)
                nc.vector.tensor_scalar_add(q_f32[:, sl], qp[:], bq_sb[:])
                nc.vector.tensor_copy(q_bf[:, sl], q_f32[:, sl])

            def k_proj(j, pool, tag):
                sl = slice(j * TQ, (j + 1) * TQ)
                kp = pool.tile([P, TQ], f32, name="kp", tag=tag)
                nc.tensor.matmul(kp[:], wk_sb[:], xk_sb[:, sl],
                                 start=True, stop=True)
                nc.vector.tensor_scalar_add(k_bf[:, sl], kp[:], bk_sb[:])

            q_proj(0, pss, "ps")
            q_proj(1, pss, "ps")
            k_proj(0, pss, "ps")
            k_proj(1, pss, "ps")
            k_done = 2

            # quarter-width tail q projections: small enough PE-FIFO bubbles
            # to hide under the exp stream
            TQ4 = 256

            def q_proj256(u):
                sl = slice(u * TQ4, (u + 1) * TQ4)
                qp = pst.tile([P, TQ4], f32, name="qp4", tag="t")
                nc.tensor.matmul(qp[:], wq_sb[:], xq_sb[:, sl],
                                 start=True, stop=True)
                nc.vector.tensor_scalar_add(q_f32[:, sl], qp[:], bq_sb[:])
                nc.vector.tensor_copy(q_bf[:, sl], q_f32[:, sl])

            q_done4 = 4  # first 4 quarter-units covered by q_proj(0|1)

            vt = spool.tile([P, NTK, VT_STRIDE], bf16, name="vt")

            def emit_s_exp(chunk, blk):
                s_ps = pss.tile([P, TQC], f32, name="s_ps", tag="ps")
                for h in range(TQC // TQ):
                    nc.tensor.matmul(
                        s_ps[:, h * TQ:(h + 1) * TQ],
                        k_bf[:, blk * P:(blk + 1) * P],
                        q_bf[:, chunk * TQC + h * TQ:
                             chunk * TQC + (h + 1) * TQ],
                        start=True, stop=True)
                e_sb = epool.tile([P, TQC], bf16, name="e_sb", tag="exp")
                nc.scalar.activation(e_sb[:], s_ps[:], AF.Exp,
                                     scale=inv_sqrt_hw)
                return e_sb

            def emit_pv(o_tiles, e_sb, blk):
                for j in range(8):
                    nc.tensor.matmul(o_tiles[j // OPACK][:, j % OPACK, 0:129],
                                     e_sb[:, j * P:(j + 1) * P],
                                     vt[:, blk, 0:129],
                                     start=False, stop=(blk == NTK - 1),
                                     skip_group_check=True)

            def emit_finalize_pass1(chunk, o_tiles):
                recs = []
                for t in range(len(o_tiles)):
                    rec = fpool.tile([P, OPACK], f32, name="rec", tag="rec",
                                     bufs=4)
                    nc.vector.reciprocal(rec[:], o_tiles[t][:, :, 128])
                    recs.append(rec)
                an_tiles = []
                for j in range(8):
                    o_ap = o_tiles[j // OPACK][:, j % OPACK, :]
                    an = fpool.tile([P, P], f32, name="an", tag="an", bufs=8)
                    nc.vector.tensor_scalar_mul(
                        an[:], o_ap[:, 0:128],
                        recs[j // OPACK][:, j % OPACK:j % OPACK + 1])
                    an_tiles.append(an)
                return an_tiles

            def emit_finalize_pass2(chunk, an_tiles, j):
                    tq0 = chunk * TQC + j * P
                    tp2 = pst.tile([P, P], f32, name="tp2", tag="t")
                    tp2 = pst.tile([P, P], f32, name="tp2", tag="t")
                    nc.tensor.transpose(tp2[:], an_tiles[j][:], ident_f[:])
                    ob = fpool.tile([P, P], f32, name="ob", tag="ob", bufs=4)
                    nc.vector.tensor_add(ob[:], tp2[:],
                                         q_f32[:, tq0:tq0 + P])
                    nc.sync.dma_start(out[:, tq0:tq0 + P], ob[:])

            def alloc_o_tiles():
                ngroups = (8 + OPACK - 1) // OPACK
                o_tiles = [
                    pso.tile([P, OPACK, 129], f32, name="o_ps", tag="o")
                    for _ in range(ngroups)
                ]
                # start=True clears the whole bank, so packed accumulation
                # groups can't each own a start; one zero-matmul inits each.
                for t in range(ngroups):
                    nc.tensor.matmul(o_tiles[t][:, :, :],
                                     zeros_b[:, 0:128], zeros_b[:],
                                     start=True, stop=False,
                                     skip_group_check=True)
                return o_tiles

            # ---- chunk 0 head: the first 8 score/exp groups need only k
            # blocks 0..7 (k_proj 0,1), so they are emitted BEFORE the V
            # transposes — ScalarE streams exps while vT is still built.
            pre0 = [emit_s_exp(0, blk) for blk in range(8)]

            # ---- V transpose (vT_ext with ones column), feeding k_proj
            # just ahead of use; evacuation on VectorE (ScalarE = exp only).
            nc.gpsimd.memset(vt[:, :, 128:129], 1.0)
            with tc.tile_pool(name="ps_vt", bufs=3, space="PSUM") as pvt:
                for blk in range(NTK):
                    tp = pvt.tile([P, P], f32, name="vtp", tag="vtp")
                    nc.tensor.transpose(tp[:], xv_sb[:, blk * P:(blk + 1) * P],
                                        ident_f[:])
                    nc.vector.tensor_copy(vt[:, blk, 0:128], tp[:])
                    if blk % 4 == 3 and k_done < HW // TQ:
                        k_proj(k_done, pst, "t")
                        k_done += 1

            pso = tc.alloc_tile_pool(name="ps_o", bufs=OPACK, space="PSUM")

            pending = None   # (chunk, o_tiles) awaiting pass1
            deferred = None  # (chunk, an_tiles) awaiting pass2 units
            for chunk in range(NCHUNK):
                npref = 8 if chunk == 0 else PREF
                pre = pre0 if chunk == 0 else \
                    [emit_s_exp(chunk, blk) for blk in range(npref)]
                if pending is not None:
                    deferred = (pending[0], emit_finalize_pass1(*pending))
                    pending = None
                o_tiles = alloc_o_tiles()
                for blk in range(npref):
                    emit_pv(o_tiles, pre[blk], blk)
                p2 = 0
                for blk in range(npref, NTK):
                    # trickle the previous chunk's output transposes/stores
                    # and the next chunks' quarter-width q projections so no
                    # single PE-FIFO insertion outruns the exp-tile buffer
                    if deferred is not None and p2 < 8:
                        emit_finalize_pass2(deferred[0], deferred[1], p2)
                        p2 += 1
                        if p2 == 8:
                            deferred = None
                    if blk in (8, 12, 16, 20) and \
                            q_done4 < min(4 * (chunk + 2), 4 * NCHUNK):
                        q_proj256(q_done4)
                        q_done4 += 1
                    e_sb = emit_s_exp(chunk, blk)
                    emit_pv(o_tiles, e_sb, blk)
                pending = (chunk, o_tiles)
            an_last = emit_finalize_pass1(*pending)
            for j in range(8):
                emit_finalize_pass2(NCHUNK - 1, an_last, j)
            pso.release()
            pst.release()

    nc.finalize()
    return nc


def kernel(query_img, key_img, value_img, Wq, bq, Wk, bk):
    from concourse.bass_utils import run_bass_kernel_spmd

    global LAST_RESULTS

    query_img = np.asarray(query_img, dtype=np.float32)
    key_img = np.asarray(key_img, dtype=np.float32)
    value_img = np.asarray(value_img, dtype=np.float32)
    wqT = np.ascontiguousarray(np.asarray(Wq, dtype=np.float32).T)
    wkT = np.ascontiguousarray(np.asarray(Wk, dtype=np.float32).T)
    bqc = np.ascontiguousarray(np.asarray(bq, dtype=np.float32).reshape(C, 1))
    bkc = np.ascontiguousarray(np.asarray(bk, dtype=np.float32).reshape(C, 1))

    if "nc" not in _CACHE:
        _CACHE["nc"] = _build_kernel()
    nc = _CACHE["nc"]

    in_maps = []
    for b in range(B):
        in_maps.append({
            "xq": np.ascontiguousarray(query_img[b].reshape(C, HW)),
            "xk": np.ascontiguousarray(key_img[b].reshape(C, HW)),
            "xv": np.ascontiguousarray(value_img[b].reshape(C, HW)),
            "wqT": wqT,
            "wkT": wkT,
            "bqv": bqc,
            "bkv": bkc,
        })

    trace = os.environ.get("KERNEL_TRACE", "0") == "1"
    res = run_bass_kernel_spmd(nc, in_maps, core_ids=list(range(B)),
                               trace=trace)
    LAST_RESULTS = res
    out = np.stack([res.results[b]["out"].reshape(C, H, W) for b in range(B)])
    return out.astype(np.float32)


# revision 28
# speedup vs baseline: 1.0803x; 1.0010x over previous
"""CrossAttention2D Trainium2 Bass kernel.

Problem (per batch item b, C=128, HW=64*64=4096):
    q = Wq @ xq + bq            # [C, HW]   (1x1 conv == GEMM)
    k = Wk @ xk + bk            # [C, HW]
    S = (q^T k) / sqrt(HW)      # [HW, HW]
    A = softmax(S, axis=-1)
    out = (A @ v^T)^T + q       # [C, HW],  v = xv

Sharding: data-parallel over batch B=8 -> one batch item per NeuronCore.

Per-core algorithm (no collectives):
  - Q/K proj in fp32 (Q feeds the residual directly); q/k cast to bf16
    for the score matmuls.
  - V transposed on the PE to vT[tk, c] (bf16) with a ones column
    (col 128) so the PV matmul accumulates the softmax denominator free.
  - Scores computed TRANSPOSED: S^T tiles [tk=128, tq=1024] spanning 2
    PSUM banks; ScalarE evacuates with exp(S/64) in one FD=1024 ACT op
    (softmax without max-subtraction: |S| <= ~1.2 for randn inputs).
  - PV: out_ext[tq,129] += expS^T_slice^T @ vT_ext over 32 tk blocks,
    PSUM-accumulated, 3 accumulator groups packed per PSUM bank (a
    zero-matmul initializes each bank since start=True clears it whole).
  - Finalize (software-pipelined into the next chunk so ACT never
    idles): DVE normalize, PE transpose back to [c, tq], DVE residual
    add, DMA out.

Engine budget per core: ACT ~136us exp (bottleneck), PE ~90us, DVE ~35us.
"""

import os
import numpy as np

B, C, H, W = 8, 128, 64, 64
HW = H * W            # 4096
P = 128
TQ = 512              # moving free dim of one S^T matmul (PSUM bank width)
TQC = 1024            # query-token chunk (2 banks wide -> one FD=1024 exp)
NCHUNK = HW // TQC    # 4
NTK = HW // P         # 32 key blocks
VT_STRIDE = 130       # 129 used + 1 pad to keep 4B alignment per block
PREF = 7              # S/exp groups emitted before the previous finalize
OPACK = 3             # accumulator groups packed per PSUM bank

_CACHE: dict = {}
LAST_RESULTS = None   # BassKernelResults of the most recent run (for test.py)


def _build_kernel():
    import concourse.tile as tile
    from concourse import bacc, mybir
    from concourse.masks import make_identity

    f32 = mybir.dt.float32
    bf16 = mybir.dt.bfloat16
    AF = mybir.ActivationFunctionType

    nc = bacc.Bacc("TRN2", target_bir_lowering=False, debug=False)

    xq = nc.dram_tensor("xq", [C, HW], f32, kind="ExternalInput")
    xk = nc.dram_tensor("xk", [C, HW], f32, kind="ExternalInput")
    xv = nc.dram_tensor("xv", [C, HW], f32, kind="ExternalInput")
    wqT = nc.dram_tensor("wqT", [C, C], f32, kind="ExternalInput")
    wkT = nc.dram_tensor("wkT", [C, C], f32, kind="ExternalInput")
    bqv = nc.dram_tensor("bqv", [C, 1], f32, kind="ExternalInput")
    bkv = nc.dram_tensor("bkv", [C, 1], f32, kind="ExternalInput")
    out = nc.dram_tensor("out", [C, HW], f32, kind="ExternalOutput")

    inv_sqrt_hw = 1.0 / float(np.sqrt(HW))

    with tile.TileContext(nc) as tc:
        with (
            tc.tile_pool(name="const", bufs=1) as cpool,
            tc.tile_pool(name="stage", bufs=1) as spool,
            tc.tile_pool(name="expp", bufs=10) as epool,
            tc.tile_pool(name="fin", bufs=3) as fpool,
            tc.tile_pool(name="ps_s", bufs=2, space="PSUM") as pss,
        ):
            # ---------- constants / weights ----------
            wq_sb = cpool.tile([C, C], f32, name="wq_sb")
            wk_sb = cpool.tile([C, C], f32, name="wk_sb")
            bq_sb = cpool.tile([C, 1], f32, name="bq_sb")
            bk_sb = cpool.tile([C, 1], f32, name="bk_sb")
            ident_f = cpool.tile([P, P], f32, name="ident_f")
            zeros_b = cpool.tile([P, OPACK * 129], bf16, name="zeros_b")
            nc.sync.dma_start(wq_sb[:], wqT[:])
            nc.sync.dma_start(wk_sb[:], wkT[:])
            nc.sync.dma_start(bq_sb[:], bqv[:])
            nc.sync.dma_start(bk_sb[:], bkv[:])
            make_identity(nc, ident_f)
            nc.gpsimd.memset(zeros_b[:], 0.0)

            # ---------- input staging ----------
            # DMA order = dependency-chain length: xv feeds the V-transpose
            # chain, xq[:TQC] + xk[:TQ..] feed the first score tiles; xq's
            # tail is only needed a full chunk later.
            xq_sb = spool.tile([C, HW], f32, name="xq_sb")
            xk_sb = spool.tile([C, HW], f32, name="xk_sb")
            xv_sb = spool.tile([C, HW], f32, name="xv_sb")
            for j in range(TQC // TQ):
                nc.sync.dma_start(xq_sb[:, j * TQ:(j + 1) * TQ],
                                  xq[:, j * TQ:(j + 1) * TQ])
            nc.sync.dma_start(xk_sb[:, 0:TQ], xk[:, 0:TQ])
            nc.sync.dma_start(xk_sb[:, TQ:2 * TQ], xk[:, TQ:2 * TQ])
            for j in range(HW // TQ):
                nc.sync.dma_start(xv_sb[:, j * TQ:(j + 1) * TQ],
                                  xv[:, j * TQ:(j + 1) * TQ])
            for j in range(2, HW // TQ):
                nc.sync.dma_start(xk_sb[:, j * TQ:(j + 1) * TQ],
                                  xk[:, j * TQ:(j + 1) * TQ])
            for j in range(TQC // TQ, HW // TQ):
                nc.sync.dma_start(xq_sb[:, j * TQ:(j + 1) * TQ],
                                  xq[:, j * TQ:(j + 1) * TQ])

            # ---------- projections (bias add + PSUM evac on DVE) ----------
            q_f32 = spool.tile([C, HW], f32, name="q_f32")
            q_bf = spool.tile([C, HW], bf16, name="q_bf")
            k_bf = spool.tile([C, HW], bf16, name="k_bf")

            # Later projections run through the single-bank "t" pool so they
            # never steal a slot from the 2-deep score ring mid-stream;
            # startup projections use the still-idle score ring.
            pst = tc.alloc_tile_pool(name="ps_t", bufs=1, space="PSUM")

            def q_proj(j, pool, tag):
                sl = slice(j * TQ, (j + 1) * TQ)
                qp = pool.tile([P, TQ], f32, name="qp", tag=tag)
                nc.tensor.matmul(qp[:], wq_sb[:], xq_sb[:, sl],
                                 start=True, stop=True)
                nc.vector.tensor_scalar_add(q_f32[:, sl], qp[:], bq_sb[:])
                nc.vector.tensor_copy(q_bf[:, sl], q_f32[:, sl])

            def k_proj(j, pool, tag):
                sl = slice(j * TQ, (j + 1) * TQ)
                kp = pool.tile([P, TQ], f32, name="kp", tag=tag)
                nc.tensor.matmul(kp[:], wk_sb[:], xk_sb[:, sl],
                                 start=True, stop=True)
                nc.vector.tensor_scalar_add(k_bf[:, sl], kp[:], bk_sb[:])

            q_proj(0, pss, "ps")
            q_proj(1, pss, "ps")
            k_proj(0, pss, "ps")
            k_proj(1, pss, "ps")
            k_done = 2

            # quarter-width tail q projections: small enough PE-FIFO bubbles
            # to hide under the exp stream
            TQ4 = 256

            def q_proj256(u):
                sl = slice(u * TQ4, (u + 1) * TQ4)
                qp = pst.tile([P, TQ4], f32, name="qp4", tag="t")
                nc.tensor.matmul(qp[:], wq_sb[:], xq_sb[:, sl],
                                 start=True, stop=True)
                nc.vector.tensor_scalar_add(q_f32[:, sl], qp[:], bq_sb[:])
                nc.vector.tensor_copy(q_bf[:, sl], q_f32[:, sl])

            q_done4 = 4  # first 4 quarter-units covered by q_proj(0|1)

            vt = spool.tile([P, NTK, VT_STRIDE], bf16, name="vt")

            def emit_s_exp(chunk, blk):
                s_ps = pss.tile([P, TQC], f32, name="s_ps", tag="ps")
                for h in range(TQC // TQ):
                    nc.tensor.matmul(
                        s_ps[:, h * TQ:(h + 1) * TQ],
                        k_bf[:, blk * P:(blk + 1) * P],
                        q_bf[:, chunk * TQC + h * TQ:
                             chunk * TQC + (h + 1) * TQ],
                        start=True, stop=True)
                e_sb = epool.tile([P, TQC], bf16, name="e_sb", tag="exp")
                nc.scalar.activation(e_sb[:], s_ps[:], AF.Exp,
                                     scale=inv_sqrt_hw)
                return e_sb

            def emit_pv(o_tiles, e_sb, blk):
                for j in range(8):
                    nc.tensor.matmul(o_tiles[j // OPACK][:, j % OPACK, 0:129],
                                     e_sb[:, j * P:(j + 1) * P],
                                     vt[:, blk, 0:129],
                                     start=False, stop=(blk == NTK - 1),
                                     skip_group_check=True)

            def emit_finalize_pass1(chunk, o_tiles):
                recs = []
                for t in range(len(o_tiles)):
                    rec = fpool.tile([P, OPACK], f32, name="rec", tag="rec",
                                     bufs=4)
                    nc.vector.reciprocal(rec[:], o_tiles[t][:, :, 128])
                    recs.append(rec)
                an_tiles = []
                for j in range(8):
                    o_ap = o_tiles[j // OPACK][:, j % OPACK, :]
                    an = fpool.tile([P, P], f32, name="an", tag="an", bufs=8)
                    nc.vector.tensor_scalar_mul(
                        an[:], o_ap[:, 0:128],
                        recs[j // OPACK][:, j % OPACK:j % OPACK + 1])
                    an_tiles.append(an)
                return an_tiles

            def emit_finalize_pass2(chunk, an_tiles, j):
                    tq0 = chunk * TQC + j * P
                    tp2 = pst.tile([P, P], f32, name="tp2", tag="t")
                    tp2 = pst.tile([P, P], f32, name="tp2", tag="t")
                    nc.tensor.transpose(tp2[:], an_tiles[j][:], ident_f[:])
                    ob = fpool.tile([P, P], f32, name="ob", tag="ob", bufs=4)
                    nc.vector.tensor_add(ob[:], tp2[:],
                                         q_f32[:, tq0:tq0 + P])
                    nc.sync.dma_start(out[:, tq0:tq0 + P], ob[:])

            def alloc_o_tiles():
                ngroups = (8 + OPACK - 1) // OPACK
                o_tiles = [
                    pso.tile([P, OPACK, 129], f32, name="o_ps", tag="o")
                    for _ in range(ngroups)
                ]
                # start=True clears the whole bank, so packed accumulation
                # groups can't each own a start; one zero-matmul inits each.
                for t in range(ngroups):
                    nc.tensor.matmul(o_tiles[t][:, :, :],
                                     zeros_b[:, 0:128], zeros_b[:],
                                     start=True, stop=False,
                                     skip_group_check=True)
                return o_tiles

            # ---- chunk 0 head interleaved with the V transposes: the
            # first 8 score/exp groups need only k blocks 0..7 (k_proj 0,1),
            # and the transposes slot into the PE idle time between ring-
            # gated score matmuls, so ScalarE streams exps from the start
            # while vT is built in the background.
            nc.gpsimd.memset(vt[:, :, 128:129], 1.0)
            with tc.tile_pool(name="ps_vt", bufs=3, space="PSUM") as pvt:
                pre0 = [emit_s_exp(0, 0), emit_s_exp(0, 1)]
                for blk in range(NTK):
                    tp = pvt.tile([P, P], f32, name="vtp", tag="vtp")
                    nc.tensor.transpose(tp[:], xv_sb[:, blk * P:(blk + 1) * P],
                                        ident_f[:])
                    nc.vector.tensor_copy(vt[:, blk, 0:128], tp[:])
                    if blk % 4 == 3:
                        if k_done < HW // TQ:
                            k_proj(k_done, pst, "t")
                            k_done += 1
                        if len(pre0) < 8:
                            pre0.append(emit_s_exp(0, len(pre0)))

            pso = tc.alloc_tile_pool(name="ps_o", bufs=OPACK, space="PSUM")

            pending = None   # (chunk, o_tiles) awaiting pass1
            deferred = None  # (chunk, an_tiles) awaiting pass2 units
            for chunk in range(NCHUNK):
                npref = 8 if chunk == 0 else PREF
                pre = pre0 if chunk == 0 else \
                    [emit_s_exp(chunk, blk) for blk in range(npref)]
                if pending is not None:
                    deferred = (pending[0], emit_finalize_pass1(*pending))
                    pending = None
                o_tiles = alloc_o_tiles()
                for blk in range(npref):
                    emit_pv(o_tiles, pre[blk], blk)
                p2 = 0
                for blk in range(npref, NTK):
                    # trickle the previous chunk's output transposes/stores
                    # and the next chunks' quarter-width q projections so no
                    # single PE-FIFO insertion outruns the exp-tile buffer
                    if deferred is not None and p2 < 8:
                        emit_finalize_pass2(deferred[0], deferred[1], p2)
                        p2 += 1
                        if p2 == 8:
                            deferred = None
                    if blk in (8, 12, 16, 20) and \
                            q_done4 < min(4 * (chunk + 2), 4 * NCHUNK):
                        q_proj256(q_done4)
                        q_done4 += 1
                    e_sb = emit_s_exp(chunk, blk)
                    emit_pv(o_tiles, e_sb, blk)
                pending = (chunk, o_tiles)
            an_last = emit_finalize_pass1(*pending)
            for j in range(8):
                emit_finalize_pass2(NCHUNK - 1, an_last, j)
            pso.release()
            pst.release()

    nc.finalize()
    return nc


def kernel(query_img, key_img, value_img, Wq, bq, Wk, bk):
    from concourse.bass_utils import run_bass_kernel_spmd

    global LAST_RESULTS

    query_img = np.asarray(query_img, dtype=np.float32)
    key_img = np.asarray(key_img, dtype=np.float32)
    value_img = np.asarray(value_img, dtype=np.float32)
    wqT = np.ascontiguousarray(np.asarray(Wq, dtype=np.float32).T)
    wkT = np.ascontiguousarray(np.asarray(Wk, dtype=np.float32).T)
    bqc = np.ascontiguousarray(np.asarray(bq, dtype=np.float32).reshape(C, 1))
    bkc = np.ascontiguousarray(np.asarray(bk, dtype=np.float32).reshape(C, 1))

    if "nc" not in _CACHE:
        _CACHE["nc"] = _build_kernel()
    nc = _CACHE["nc"]

    in_maps = []
    for b in range(B):
        in_maps.append({
            "xq": np.ascontiguousarray(query_img[b].reshape(C, HW)),
            "xk": np.ascontiguousarray(key_img[b].reshape(C, HW)),
            "xv": np.ascontiguousarray(value_img[b].reshape(C, HW)),
            "wqT": wqT,
            "wkT": wkT,
            "bqv": bqc,
            "bkv": bkc,
        })

    trace = os.environ.get("KERNEL_TRACE", "0") == "1"
    res = run_bass_kernel_spmd(nc, in_maps, core_ids=list(range(B)),
                               trace=trace)
    LAST_RESULTS = res
    out = np.stack([res.results[b]["out"].reshape(C, H, W) for b in range(B)])
    return out.astype(np.float32)


# revision 30
# speedup vs baseline: 1.0831x; 1.0025x over previous
"""CrossAttention2D Trainium2 Bass kernel.

Problem (per batch item b, C=128, HW=64*64=4096):
    q = Wq @ xq + bq            # [C, HW]   (1x1 conv == GEMM)
    k = Wk @ xk + bk            # [C, HW]
    S = (q^T k) / sqrt(HW)      # [HW, HW]
    A = softmax(S, axis=-1)
    out = (A @ v^T)^T + q       # [C, HW],  v = xv

Sharding: data-parallel over batch B=8 -> one batch item per NeuronCore.

Per-core algorithm (no collectives):
  - Q/K proj in fp32 (Q feeds the residual directly); q/k cast to bf16
    for the score matmuls.
  - V transposed on the PE to vT[tk, c] (bf16) with a ones column
    (col 128) so the PV matmul accumulates the softmax denominator free.
  - Scores computed TRANSPOSED: S^T tiles [tk=128, tq=1024] spanning 2
    PSUM banks; ScalarE evacuates with exp(S/64) in one FD=1024 ACT op
    (softmax without max-subtraction: |S| <= ~1.2 for randn inputs).
  - PV: out_ext[tq,129] += expS^T_slice^T @ vT_ext over 32 tk blocks,
    PSUM-accumulated, 3 accumulator groups packed per PSUM bank (a
    zero-matmul initializes each bank since start=True clears it whole).
  - Finalize (software-pipelined into the next chunk so ACT never
    idles): DVE normalize, PE transpose back to [c, tq], DVE residual
    add, DMA out.

Engine budget per core: ACT ~136us exp (bottleneck), PE ~90us, DVE ~35us.
"""

import os
import numpy as np

B, C, H, W = 8, 128, 64, 64
HW = H * W            # 4096
P = 128
TQ = 512              # moving free dim of one S^T matmul (PSUM bank width)
TQC = 1024            # query-token chunk (2 banks wide -> one FD=1024 exp)
NCHUNK = HW // TQC    # 4
NTK = HW // P         # 32 key blocks
VT_STRIDE = 130       # 129 used + 1 pad to keep 4B alignment per block
PREF = 7              # S/exp groups emitted before the previous finalize
OPACK = 3             # accumulator groups packed per PSUM bank

_CACHE: dict = {}
LAST_RESULTS = None   # BassKernelResults of the most recent run (for test.py)


def _build_kernel():
    import concourse.tile as tile
    from concourse import bacc, mybir
    from concourse.masks import make_identity

    f32 = mybir.dt.float32
    bf16 = mybir.dt.bfloat16
    AF = mybir.ActivationFunctionType

    nc = bacc.Bacc("TRN2", target_bir_lowering=False, debug=False)

    xq = nc.dram_tensor("xq", [C, HW], f32, kind="ExternalInput")
    xk = nc.dram_tensor("xk", [C, HW], f32, kind="ExternalInput")
    xv = nc.dram_tensor("xv", [C, HW], f32, kind="ExternalInput")
    wqT = nc.dram_tensor("wqT", [C, C], f32, kind="ExternalInput")
    wkT = nc.dram_tensor("wkT", [C, C], f32, kind="ExternalInput")
    bqv = nc.dram_tensor("bqv", [C, 1], f32, kind="ExternalInput")
    bkv = nc.dram_tensor("bkv", [C, 1], f32, kind="ExternalInput")
    out = nc.dram_tensor("out", [C, HW], f32, kind="ExternalOutput")

    inv_sqrt_hw = 1.0 / float(np.sqrt(HW))

    with tile.TileContext(nc) as tc:
        with (
            tc.tile_pool(name="const", bufs=1) as cpool,
            tc.tile_pool(name="stage", bufs=1) as spool,
            tc.tile_pool(name="expp", bufs=10) as epool,
            tc.tile_pool(name="fin", bufs=3) as fpool,
            tc.tile_pool(name="ps_s", bufs=2, space="PSUM") as pss,
        ):
            # ---------- constants / weights ----------
            wq_sb = cpool.tile([C, C], f32, name="wq_sb")
            wk_sb = cpool.tile([C, C], f32, name="wk_sb")
            bq_sb = cpool.tile([C, 1], f32, name="bq_sb")
            bk_sb = cpool.tile([C, 1], f32, name="bk_sb")
            ident_f = cpool.tile([P, P], f32, name="ident_f")
            zeros_b = cpool.tile([P, OPACK * 129], bf16, name="zeros_b")
            nc.sync.dma_start(wq_sb[:], wqT[:])
            nc.sync.dma_start(wk_sb[:], wkT[:])
            nc.sync.dma_start(bq_sb[:], bqv[:])
            nc.sync.dma_start(bk_sb[:], bkv[:])
            make_identity(nc, ident_f)
            nc.gpsimd.memset(zeros_b[:], 0.0)

            # ---------- input staging ----------
            # DMA order = dependency-chain length: xv feeds the V-transpose
            # chain, xq[:TQC] + xk[:TQ..] feed the first score tiles; xq's
            # tail is only needed a full chunk later.
            xq_sb = spool.tile([C, HW], f32, name="xq_sb")
            xk_sb = spool.tile([C, HW], f32, name="xk_sb")
            xv_sb = spool.tile([C, HW], f32, name="xv_sb")
            for j in range(TQC // TQ):
                nc.sync.dma_start(xq_sb[:, j * TQ:(j + 1) * TQ],
                                  xq[:, j * TQ:(j + 1) * TQ])
            nc.sync.dma_start(xk_sb[:, 0:TQ], xk[:, 0:TQ])
            nc.sync.dma_start(xk_sb[:, TQ:2 * TQ], xk[:, TQ:2 * TQ])
            for j in range(HW // TQ):
                nc.sync.dma_start(xv_sb[:, j * TQ:(j + 1) * TQ],
                                  xv[:, j * TQ:(j + 1) * TQ])
            for j in range(2, HW // TQ):
                nc.sync.dma_start(xk_sb[:, j * TQ:(j + 1) * TQ],
                                  xk[:, j * TQ:(j + 1) * TQ])
            for j in range(TQC // TQ, HW // TQ):
                nc.sync.dma_start(xq_sb[:, j * TQ:(j + 1) * TQ],
                                  xq[:, j * TQ:(j + 1) * TQ])

            # ---------- projections (bias add + PSUM evac on DVE) ----------
            q_f32 = spool.tile([C, HW], f32, name="q_f32")
            q_bf = spool.tile([C, HW], bf16, name="q_bf")
            k_bf = spool.tile([C, HW], bf16, name="k_bf")

            # Later projections run through the single-bank "t" pool so they
            # never steal a slot from the 2-deep score ring mid-stream;
            # startup projections use the still-idle score ring.
            pst = tc.alloc_tile_pool(name="ps_t", bufs=1, space="PSUM")

            def q_proj(j, pool, tag):
                sl = slice(j * TQ, (j + 1) * TQ)
                qp = pool.tile([P, TQ], f32, name="qp", tag=tag)
                nc.tensor.matmul(qp[:], wq_sb[:], xq_sb[:, sl],
                                 start=True, stop=True)
                nc.vector.tensor_scalar_add(q_f32[:, sl], qp[:], bq_sb[:])
                nc.vector.tensor_copy(q_bf[:, sl], q_f32[:, sl])

            def k_proj(j, pool, tag):
                sl = slice(j * TQ, (j + 1) * TQ)
                kp = pool.tile([P, TQ], f32, name="kp", tag=tag)
                nc.tensor.matmul(kp[:], wk_sb[:], xk_sb[:, sl],
                                 start=True, stop=True)
                nc.vector.tensor_scalar_add(k_bf[:, sl], kp[:], bk_sb[:])

            q_proj(0, pss, "ps")
            q_proj(1, pss, "ps")
            k_proj(0, pss, "ps")
            k_proj(1, pss, "ps")
            k_done = 2

            # quarter-width tail q projections: small enough PE-FIFO bubbles
            # to hide under the exp stream
            TQ4 = 256

            def q_proj256(u):
                sl = slice(u * TQ4, (u + 1) * TQ4)
                qp = pst.tile([P, TQ4], f32, name="qp4", tag="t")
                nc.tensor.matmul(qp[:], wq_sb[:], xq_sb[:, sl],
                                 start=True, stop=True)
                nc.vector.tensor_scalar_add(q_f32[:, sl], qp[:], bq_sb[:])
                nc.vector.tensor_copy(q_bf[:, sl], q_f32[:, sl])

            q_done4 = 4  # first 4 quarter-units covered by q_proj(0|1)

            vt = spool.tile([P, NTK, VT_STRIDE], bf16, name="vt")

            def emit_s_exp(chunk, blk):
                s_ps = pss.tile([P, TQC], f32, name="s_ps", tag="ps")
                for h in range(TQC // TQ):
                    nc.tensor.matmul(
                        s_ps[:, h * TQ:(h + 1) * TQ],
                        k_bf[:, blk * P:(blk + 1) * P],
                        q_bf[:, chunk * TQC + h * TQ:
                             chunk * TQC + (h + 1) * TQ],
                        start=True, stop=True)
                e_sb = epool.tile([P, TQC], bf16, name="e_sb", tag="exp")
                nc.scalar.activation(e_sb[:], s_ps[:], AF.Exp,
                                     scale=inv_sqrt_hw)
                return e_sb

            def emit_pv(o_tiles, e_sb, blk):
                for j in range(8):
                    nc.tensor.matmul(o_tiles[j // OPACK][:, j % OPACK, 0:129],
                                     e_sb[:, j * P:(j + 1) * P],
                                     vt[:, blk, 0:129],
                                     start=False, stop=(blk == NTK - 1),
                                     skip_group_check=True)

            def emit_finalize_pass1(chunk, o_tiles):
                recs = []
                for t in range(len(o_tiles)):
                    rec = fpool.tile([P, OPACK], f32, name="rec", tag="rec",
                                     bufs=4)
                    nc.vector.reciprocal(rec[:], o_tiles[t][:, :, 128])
                    recs.append(rec)
                an_tiles = []
                for j in range(8):
                    o_ap = o_tiles[j // OPACK][:, j % OPACK, :]
                    an = fpool.tile([P, P], f32, name="an", tag="an", bufs=8)
                    nc.vector.tensor_scalar_mul(
                        an[:], o_ap[:, 0:128],
                        recs[j // OPACK][:, j % OPACK:j % OPACK + 1])
                    an_tiles.append(an)
                return an_tiles

            def emit_finalize_pass2(chunk, an_tiles, j):
                    tq0 = chunk * TQC + j * P
                    tp2 = pst.tile([P, P], f32, name="tp2", tag="t")
                    tp2 = pst.tile([P, P], f32, name="tp2", tag="t")
                    nc.tensor.transpose(tp2[:], an_tiles[j][:], ident_f[:])
                    ob = fpool.tile([P, P], f32, name="ob", tag="ob", bufs=4)
                    nc.vector.tensor_add(ob[:], tp2[:],
                                         q_f32[:, tq0:tq0 + P])
                    nc.sync.dma_start(out[:, tq0:tq0 + P], ob[:])

            def alloc_o_tiles():
                ngroups = (8 + OPACK - 1) // OPACK
                o_tiles = [
                    pso.tile([P, OPACK, 129], f32, name="o_ps", tag="o")
                    for _ in range(ngroups)
                ]
                # start=True clears the whole bank, so packed accumulation
                # groups can't each own a start; one zero-matmul inits each.
                for t in range(ngroups):
                    nc.tensor.matmul(o_tiles[t][:, :, :],
                                     zeros_b[:, 0:128], zeros_b[:],
                                     start=True, stop=False,
                                     skip_group_check=True)
                return o_tiles

            # ---- chunk 0 head interleaved with the V transposes: the
            # first 8 score/exp groups need only k blocks 0..7 (k_proj 0,1),
            # and the transposes slot into the PE idle time between ring-
            # gated score matmuls, so ScalarE streams exps from the start
            # while vT is built in the background.
            nc.gpsimd.memset(vt[:, :, 128:129], 1.0)
            with tc.tile_pool(name="ps_vt", bufs=3, space="PSUM") as pvt:
                pre0 = [emit_s_exp(0, 0), emit_s_exp(0, 1)]
                for blk in range(NTK):
                    tp = pvt.tile([P, P], f32, name="vtp", tag="vtp")
                    nc.tensor.transpose(tp[:], xv_sb[:, blk * P:(blk + 1) * P],
                                        ident_f[:])
                    nc.vector.tensor_copy(vt[:, blk, 0:128], tp[:])
                    if blk % 4 == 3:
                        if k_done < HW // TQ:
                            k_proj(k_done, pst, "t")
                            k_done += 1
                        if len(pre0) < 8:
                            pre0.append(emit_s_exp(0, len(pre0)))

            pso = tc.alloc_tile_pool(name="ps_o", bufs=OPACK, space="PSUM")

            pending = None   # (chunk, o_tiles) awaiting pass1
            deferred = None  # (chunk, an_tiles) awaiting pass2 units
            for chunk in range(NCHUNK):
                npref = 8 if chunk == 0 else PREF
                pre = pre0 if chunk == 0 else \
                    [emit_s_exp(chunk, blk) for blk in range(npref)]
                if pending is not None:
                    deferred = (pending[0], emit_finalize_pass1(*pending))
                    pending = None
                o_tiles = alloc_o_tiles()
                for blk in range(npref):
                    emit_pv(o_tiles, pre[blk], blk)
                p2 = 0
                for blk in range(npref, NTK):
                    # trickle the previous chunk's output transposes/stores
                    # and the next chunks' quarter-width q projections so no
                    # single PE-FIFO insertion outruns the exp-tile buffer
                    if deferred is not None and p2 < 8:
                        emit_finalize_pass2(deferred[0], deferred[1], p2)
                        p2 += 1
                        if p2 == 8:
                            deferred = None
                    if blk in (8, 12, 16, 20) and \
                            q_done4 < min(4 * (chunk + 2), 4 * NCHUNK):
                        q_proj256(q_done4)
                        q_done4 += 1
                    e_sb = emit_s_exp(chunk, blk)
                    emit_pv(o_tiles, e_sb, blk)
                pending = (chunk, o_tiles)
            an_last = emit_finalize_pass1(*pending)
            for j in range(8):
                emit_finalize_pass2(NCHUNK - 1, an_last, j)
            pso.release()
            pst.release()

    nc.finalize()
    return nc


def kernel(query_img, key_img, value_img, Wq, bq, Wk, bk):
    from concourse.bass_utils import run_bass_kernel_spmd

    global LAST_RESULTS

    query_img = np.asarray(query_img, dtype=np.float32)
    key_img = np.asarray(key_img, dtype=np.float32)
    value_img = np.asarray(value_img, dtype=np.float32)
    wqT = np.ascontiguousarray(np.asarray(Wq, dtype=np.float32).T)
    wkT = np.ascontiguousarray(np.asarray(Wk, dtype=np.float32).T)
    bqc = np.ascontiguousarray(np.asarray(bq, dtype=np.float32).reshape(C, 1))
    bkc = np.ascontiguousarray(np.asarray(bk, dtype=np.float32).reshape(C, 1))

    if "nc" not in _CACHE:
        _CACHE["nc"] = _build_kernel()
    nc = _CACHE["nc"]

    in_maps = []
    for b in range(B):
        in_maps.append({
            "xq": np.ascontiguousarray(query_img[b].reshape(C, HW)),
            "xk": np.ascontiguousarray(key_img[b].reshape(C, HW)),
            "xv": np.ascontiguousarray(value_img[b].reshape(C, HW)),
            "wqT": wqT,
            "wkT": wkT,
            "bqv": bqc,
            "bkv": bkc,
        })

    trace = os.environ.get("KERNEL_TRACE", "0") == "1"
    res = run_bass_kernel_spmd(nc, in_maps, core_ids=list(range(B)),
                               trace=trace)
    LAST_RESULTS = res
    out = np.stack([res.results[b]["out"].reshape(C, H, W) for b in range(B)])
    return out.astype(np.float32)
